# revision 3
# baseline (speedup 1.0000x reference)
"""Trainium2 Bass kernel for nn_AdaptiveConv2 — v2 (pixel-major convs).

Data-parallel over batch: 8 images -> 8 NeuronCores, no collectives.

Cost-model-driven design (matmul cost = output free size; stationary loads
free):
  - Convs pixel-major: stationary = activation slices [K=(row-parity,ch),
    M=128 px of one image row], moving = weights [K, 64] -> N=64/matmul.
    7 matmuls per output row (6 tap-reads + 1 K=1 bias read). Activations
    stored channel-major in (row-parity, channel) partition layout
    [128, 66 row-pair groups, 130 cols]; one PE transpose per row-pair
    returns the pixel-major tanh output to that layout, one DVE copy per
    8 rows commits it.
  - g (depthwise basis conv of x): rank-1 im2col — x replicated x9 taps on
    partitions (host-prepped, DMA-windowed); 8 matmuls of N=48 per row
    cover all 9 taps at once. Lands pixel-major in PSUM; two strided
    copies (ACT+DVE) pack bf16 (kp, c, kk) for the epilogue.
  - Epilogue per row on DVE: 3 broadcast products (2x packed) + 3 tree
    adds; PE transposes to channel order, ACT/Pool cast f32, 1 DMA/row.
"""

import os
import sys

sys.path.insert(0, "/opt/trn_rl_repo")

if os.environ.get("JAX_PLATFORMS") and "axon" not in os.environ["JAX_PLATFORMS"]:
    if "jax" not in sys.modules:
        del os.environ["JAX_PLATFORMS"]

import numpy as np
import ml_dtypes

BF16 = ml_dtypes.bfloat16
EPS = 1e-5

C = 64
H = W = 128
NPIX = H * W
NG = 66        # row-pair groups incl top/bottom halo
PW = 130       # padded width
FEAT = 6
NB = 6
OC = FEAT * NB           # 36
NWC = 7 * 12 * 64        # conv weight cols
NWG = 8 * 48             # g weight cols
NGRP = 16                # 8-row groups
SKEW = 1                 # wave skew between conv layers (same-wave emission
                         # order satisfies the one-group-ahead dependency)
EPI_W = 7                # epilogue wave offset

_CACHE = {}


def _build_graph():
    import concourse.bacc as bacc
    import concourse.bass as bass
    import concourse.tile as tile
    import concourse.mybir as mybir
    from contextlib import ExitStack

    f32 = mybir.dt.float32
    bf16 = mybir.dt.bfloat16

    nc = bacc.Bacc("TRN2", target_bir_lowering=False, debug=False, num_devices=8)

    xd_ext = nc.dram_tensor("xd", [128, NG * PW], bf16, kind="ExternalInput").ap()
    xr_ext = nc.dram_tensor("xr", [72, 8 * 16 * 8 * PW], bf16,
                            kind="ExternalInput").ap()
    wc_ext = nc.dram_tensor("wc", [128, NWC], bf16, kind="ExternalInput").ap()
    wg_ext = nc.dram_tensor("wg", [72, NWG], bf16, kind="ExternalInput").ap()
    bias_ext = nc.dram_tensor("bias", [128, 448], bf16, kind="ExternalInput").ap()
    out_ext = nc.dram_tensor("out", [C * NB, NPIX], f32, kind="ExternalOutput").ap()

    Tanh = mybir.ActivationFunctionType.Tanh
    MULT = mybir.AluOpType.mult
    ADD = mybir.AluOpType.add

    ctx = ExitStack()
    with tile.TileContext(nc) as tc, ctx:
        singles = ctx.enter_context(tc.tile_pool(name="singles", bufs=1))
        cpsum = ctx.enter_context(tc.tile_pool(name="cpsum", bufs=3, space="PSUM"))
        tpsum = ctx.enter_context(tc.tile_pool(name="tpsum", bufs=2, space="PSUM"))
        gpsum = ctx.enter_context(tc.tile_pool(name="gpsum", bufs=2, space="PSUM"))
        opsum = ctx.enter_context(tc.tile_pool(name="opsum", bufs=1, space="PSUM"))
        pixp = ctx.enter_context(tc.tile_pool(name="pixp", bufs=3))
        gsb_pool = ctx.enter_context(tc.tile_pool(name="gsb", bufs=18))
        xrw_pool = ctx.enter_context(tc.tile_pool(name="xrw", bufs=3))
        acc_pool = ctx.enter_context(tc.tile_pool(name="acc", bufs=6))
        ofl_pool = ctx.enter_context(tc.tile_pool(name="ofl", bufs=4))

        x_t = singles.tile([128, NG, PW], bf16)
        actA = singles.tile([128, NG, PW], bf16)
        actB = singles.tile([128, NG, PW], bf16)
        bft = singles.tile([128, H, OC], bf16)
        wc = singles.tile([128, NWC], bf16)
        wg = singles.tile([72, NWG], bf16)
        btile = singles.tile([128, 448], bf16)
        ones_t = singles.tile([128, 128], bf16)
        ident = singles.tile([128, 128], bf16)
        from concourse.masks import make_identity

        make_identity(nc, ident)
        nc.vector.memset(ones_t[0:1, :], 1.0)

        # input DMAs: L0 weights first, then x in chunks so early groups land
        nc.sync.dma_start(out=wc[:, 0:768], in_=wc_ext[:, 0:768])
        nc.sync.dma_start(out=btile, in_=bias_ext)
        xd3 = xd_ext.rearrange("p (a b) -> p a b", a=6)
        x_t_flat = x_t.rearrange("p a b -> p (a b)").rearrange(
            "p (a b) -> p a b", a=6
        )
        for ch in range(6):
            nc.gpsimd.dma_start(out=x_t_flat[:, ch, :], in_=xd3[:, ch, :])
        for li in range(1, 7):
            nc.sync.dma_start(
                out=wc[:, li * 768 : (li + 1) * 768],
                in_=wc_ext[:, li * 768 : (li + 1) * 768],
            )
        nc.sync.dma_start(out=wg, in_=wg_ext)

        # zero halos once (copies only ever write G 1..64, cols 1:129)
        for buf in (actA, actB):
            nc.vector.memset(buf[:, 0, :], 0.0)
            nc.vector.memset(buf[:, NG - 1, :], 0.0)
            nc.vector.memset(buf[:, 1 : NG - 1, 0:1], 0.0)
            nc.vector.memset(buf[:, 1 : NG - 1, PW - 1 :], 0.0)

        layer_in = [x_t, actA, actB, actA, actB, actA, actB]
        layer_out = [actA, actB, actA, actB, actA, actB, None]

        xr4 = xr_ext.rearrange("p (s w q) -> p s w q", s=8, w=16)

        def emit_conv_pair(li, t, j):
            """Fine-grained: one row-pair of layer li's group t (fill cone)."""
            src = layer_in[li]
            M = 64 if li < 6 else OC
            ps = cpsum.tile([128, 2 * M], f32, tag="cps")
            for r01 in range(2):
                r = 8 * t + 2 * j + r01
                a = r // 2
                off = M * r01
                if r01 == 0:
                    reads = [(a, 0), (a + 1, 3)]
                else:
                    reads = [(a + 1, 6), (a + 2, 9)]
                first = True
                for (G, b0) in reads:
                    for dj in range(3):
                        col0 = li * 768 + (b0 + dj) * 64
                        nc.tensor.matmul(
                            ps[:, off : off + M],
                            src[0:128, G, dj : dj + 128],
                            wc[0:128, col0 : col0 + M],
                            start=first,
                            stop=False,
                        )
                        first = False
                nc.tensor.matmul(
                    ps[:, off : off + M],
                    ones_t[0:1, 0:128],
                    btile[0:1, li * 64 : li * 64 + M],
                    start=False,
                    stop=True,
                )
            ps3 = ps.rearrange("p (a b) -> p a b", a=2)
            if li < 6:
                P = pixp.tile([128, 2, 64], bf16, tag="pixPf")
                nc.scalar.activation(P, ps3, Tanh)
                dst = layer_out[li]
                T = tpsum.tile([128, 128], bf16, tag="tps")
                nc.tensor.transpose(T, P, ident)
                T3 = T.rearrange("p (a b) -> p a b", a=1)
                nc.scalar.copy(dst[:, 4 * t + 1 + j : 4 * t + 2 + j, 1:129], T3)
            else:
                nc.scalar.activation(
                    bft[:, 8 * t + 2 * j : 8 * t + 2 * j + 2, :], ps3, Tanh
                )

        def emit_conv_group(li, t):
            """Layer li, rows 8t..8t+7 (one 8-row group)."""
            src = layer_in[li]
            M = 64 if li < 6 else OC
            ps = cpsum.tile([128, 512], f32, tag="cps")
            for j8 in range(8):
                r = 8 * t + j8
                a = r // 2
                off = M * j8
                if r % 2 == 0:
                    reads = [(a, 0), (a + 1, 3)]
                else:
                    reads = [(a + 1, 6), (a + 2, 9)]
                first = True
                for (G, b0) in reads:
                    for dj in range(3):
                        col0 = li * 768 + (b0 + dj) * 64
                        nc.tensor.matmul(
                            ps[:, off : off + M],
                            src[0:128, G, dj : dj + 128],
                            wc[0:128, col0 : col0 + M],
                            start=first,
                            stop=False,
                        )
                        first = False
                nc.tensor.matmul(
                    ps[:, off : off + M],
                    ones_t[0:1, 0:128],
                    btile[0:1, li * 64 : li * 64 + M],
                    start=False,
                    stop=True,
                )
            if li < 6:
                P = pixp.tile([128, 8, 64], bf16, tag="pixP")
                ps3 = ps.rearrange("p (a b) -> p a b", a=8)
                nc.scalar.activation(P, ps3, Tanh)
                dst = layer_out[li]
                T = tpsum.tile([128, 512], bf16, tag="tps")
                for j in range(4):
                    nc.tensor.transpose(
                        T[:, j * 128 : (j + 1) * 128],
                        P[:, 2 * j : 2 * j + 2, :],
                        ident,
                    )
                T4 = T.rearrange("p (a b) -> p a b", a=4)
                nc.scalar.copy(dst[:, 4 * t + 1 : 4 * t + 5, 1:129], T4)
            else:
                ps3 = bass.AP(
                    tensor=ps.tensor,
                    offset=ps.offset,
                    ap=[ps.ap[0], [OC, 8], [1, OC]],
                )
                nc.scalar.activation(bft[:, 8 * t : 8 * t + 8, :], ps3, Tanh)

        def emit_g_row(r, xrw):
            wr = r % 8
            gps = gpsum.tile([128, 384], f32, tag="gps")
            for s in range(8):
                nc.tensor.matmul(
                    gps[:, s * 48 : (s + 1) * 48],
                    xrw[0:72, s, wr, 1:129],
                    wg[0:72, s * 48 : (s + 1) * 48],
                    start=True,
                    stop=True,
                )
            gsb = gsb_pool.tile([128, 384], bf16, tag="gsb")
            # pack (s, kp, kk, c8) psum f32 -> (kp, c=8s+c8, kk) bf16
            for kk in range(2):
                src = bass.AP(
                    tensor=gps.tensor,
                    offset=gps.offset + kk * 8,
                    ap=[gps.ap[0], [48, 8], [16, 3], [1, 8]],
                )
                dst = bass.AP(
                    tensor=gsb.tensor,
                    offset=gsb.offset + kk,
                    ap=[gsb.ap[0], [16, 8], [128, 3], [2, 8]],
                )
                nc.scalar.copy(dst, src)
            return gsb

        def emit_epilogue_row(r, gsb):
            # two rows per group run on Pool (SBUF-only ops) to offload DVE;
            # they get their own tile tags so a slow Pool row can't starve
            # the DVE rows' buffer rotation
            t8 = r // 8
            on_pool = (r % 8 == 7) or (r % 8 == 6 and t8 % 4 != 3) or (
                r % 8 == 5 and t8 >= NGRP - 2)
            ve = nc.gpsimd if on_pool else nc.vector
            sfx = "p" if on_pool else ""
            prod = acc_pool.tile([128, 3, 6, 64, 2], bf16, tag="prod" + sfx,
                                 bufs=2 if on_pool else 3)
            for kp in range(3):
                g_in = bass.AP(
                    tensor=gsb.tensor,
                    offset=gsb.offset + kp * 128,
                    ap=[gsb.ap[0], [0, 6], [2, 64], [1, 2]],
                )
                b_in = bass.AP(
                    tensor=bft.tensor,
                    offset=bft.offset + r * OC + 2 * kp,
                    ap=[bft.ap[0], [6, 6], [0, 64], [1, 2]],
                )
                ve.tensor_tensor(prod[:, kp], g_in, b_in, MULT)
            d = acc_pool.tile([128, 384, 2], bf16, tag="dsum" + sfx,
                                 bufs=2 if on_pool else 3)
            pk = prod.rearrange("p q m c b -> p q (m c) b")
            ve.tensor_tensor(d, pk[:, 0], pk[:, 1], ADD)
            ve.tensor_tensor(d, d, pk[:, 2], ADD)
            acc = acc_pool.tile([128, 384], bf16, tag="acc" + sfx,
                                 bufs=3 if on_pool else 6)
            din0 = bass.AP(tensor=d.tensor, offset=d.offset,
                           ap=[d.ap[0], [128, 6], [2, 64]])
            din1 = bass.AP(tensor=d.tensor, offset=d.offset + 1,
                           ap=[d.ap[0], [128, 6], [2, 64]])
            aout = bass.AP(tensor=acc.tensor, offset=acc.offset,
                           ap=[acc.ap[0], [1, 6], [6, 64]])
            ve.tensor_tensor(aout, din0, din1, ADD)
            return acc

        def emit_out_pair(r0, acc0, acc1):
            # two rows' channel-order results -> PE transpose -> one f32
            # cast -> one DMA covering 256 contiguous pixels per channel
            tps = opsum.tile([128, 3, 2, 128], bf16, tag="ops")
            for r01, acc in ((0, acc0), (1, acc1)):
                for j in range(3):
                    nc.tensor.transpose(
                        tps[:, j, r01, :],
                        acc[:, j * 128 : (j + 1) * 128],
                        ident,
                    )
            ofl = ofl_pool.tile([128, 3, 2, 128], f32, tag="ofl")
            nc.scalar.copy(ofl, tps)
            src_ap = bass.AP(
                tensor=ofl.tensor,
                offset=ofl.offset,
                ap=[ofl.ap[0], [256, 3], [1, 256]],
            )
            dst = bass.AP(
                tensor=out_ext.tensor,
                offset=r0 * 128,
                ap=[[NPIX, 128], [128 * NPIX, 3], [1, 256]],
            )
            nc.sync.dma_start(out=dst, in_=src_ap)

        # wavefront
        xrw_tiles = {}
        gsb_rows = {}
        NW = EPI_W + NGRP
        for w in range(NW):
            win = w - (EPI_W - 4)
            if 0 <= win < NGRP:
                xrw = xrw_pool.tile([72, 8, 8, PW], bf16, tag="xrw")
                xrwf = xrw.rearrange("p a b c -> p (a b c)")
                nc.sync.dma_start(out=xrwf, in_=xr4[:, :, win, :])
                xrw_tiles[win] = xrw
            for li in range(7):
                t = w - SKEW * li
                if 0 <= t < NGRP:
                    emit_conv_group(li, t)
            tg = w - (EPI_W - 1)
            if 0 <= tg < NGRP:
                for wr in range(8):
                    gsb_rows[8 * tg + wr] = emit_g_row(
                        8 * tg + wr, xrw_tiles[tg]
                    )
                xrw_tiles.pop(tg)
            te = w - EPI_W
            if 0 <= te < NGRP:
                accs = {}
                for wr in range(8):
                    r = 8 * te + wr
                    accs[wr] = emit_epilogue_row(r, gsb_rows.pop(r))
                    if wr % 2 == 1:
                        emit_out_pair(8 * te + wr - 1, accs[wr - 1], accs[wr])

    nc.compile()
    return nc


def _fold_bn(w, b, g, be, m, v):
    scale = g / np.sqrt(v + EPS)
    wf = w * scale[:, None, None, None]
    bf = (b - m) * scale + be
    return wf.astype(np.float32), bf.astype(np.float32)


def _prep_weights(w0, b0, g0, be0, m0, v0, wm, bm, gm, bem, mm, vm,
                  wl, bl, gl, bel, ml, vl, bases):
    wc = np.zeros((128, NWC), np.float32)
    bias = np.zeros((128, 448), np.float32)
    layers = [(w0, b0, g0, be0, m0, v0)]
    for i in range(5):
        layers.append((wm[i], bm[i], gm[i], bem[i], mm[i], vm[i]))
    layers.append((wl, bl, gl, bel, ml, vl))
    for li, (w, b, g, be, m, v) in enumerate(layers):
        wf, bf = _fold_bn(w, b, g, be, m, v)
        oc = wf.shape[0]
        # blocks (64 cols each): A(dj): even 0 / odd w[.,.,0,dj];
        # B: w[1]/w[2]; C: w[0]/w[1]; D: w[2]/0   (even=rows 0:64)
        for dj in range(3):
            cA = li * 768 + (0 + dj) * 64
            cB = li * 768 + (3 + dj) * 64
            cC = li * 768 + (6 + dj) * 64
            cD = li * 768 + (9 + dj) * 64
            wc[64:128, cA : cA + oc] = wf[:, :, 0, dj].T
            wc[0:64, cB : cB + oc] = wf[:, :, 1, dj].T
            wc[64:128, cB : cB + oc] = wf[:, :, 2, dj].T
            wc[0:64, cC : cC + oc] = wf[:, :, 0, dj].T
            wc[64:128, cC : cC + oc] = wf[:, :, 1, dj].T
            wc[0:64, cD : cD + oc] = wf[:, :, 2, dj].T
        bias[0, li * 64 : li * 64 + oc] = bf
    # g weights: row (c8*9 + l), col s*48 + kp*16 + kk*8 + c8
    wgm = np.zeros((72, NWG), np.float32)
    for kp in range(3):
        for kk in range(2):
            k = 2 * kp + kk
            for c8 in range(8):
                for l in range(9):
                    wgm[c8 * 9 + l, kp * 16 + kk * 8 + c8 :: 48] = bases[k, l]
    return wc.astype(BF16), wgm.astype(BF16), bias.astype(BF16)


def _prep_x(xn):
    # xd: [128=(r01*64+c), 66, 130]; content x[c, 2G-2+r01, col-1]
    xd = np.zeros((128, NG, PW), np.float32)
    xp = np.zeros((C, 2 * NG + 1, PW), np.float32)  # rows -2..130
    xp[:, 2 : 2 + H, 1 : 1 + W] = xn
    for r01 in range(2):
        xd[r01 * 64 : r01 * 64 + 64] = xp[:, r01 : r01 + 2 * NG : 2, :]
    # xrep: [72=(c8*9+l), s, win, wr, col] = x[8s+c8, R+di, col+dj-1]
    # with R = 8*win+wr, l = 3*(di+1)+(dj+1)
    xq = np.zeros((C, H + 2, PW + 2), np.float32)  # rows -1..128, cols -2..130
    xq[:, 1 : 1 + H, 2 : 2 + W] = xn
    xrep = np.zeros((72, 8, 16, 8, PW), np.float32)
    for di in range(-1, 2):
        for dj in range(-1, 2):
            l = 3 * (di + 1) + (dj + 1)
            sl = xq[:, 1 + di : 1 + di + H, 1 + dj : 1 + dj + PW]
            for s in range(8):
                xrep[np.arange(8) * 9 + l, s] = sl[8 * s : 8 * s + 8].reshape(
                    8, 16, 8, PW
                )
    return (
        xd.reshape(128, NG * PW).astype(BF16),
        xrep.reshape(72, 8 * 16 * 8 * PW).astype(BF16),
    )


def get_nc():
    if "nc" not in _CACHE:
        _CACHE["nc"] = _build_graph()
    return _CACHE["nc"]


def kernel(**inputs):
    from concourse.bass_utils import run_bass_kernel_spmd

    nc = get_nc()
    x = np.asarray(inputs["x"], np.float32)
    wc, wgm, bias = _prep_weights(
        *[np.asarray(inputs[k], np.float32) for k in
          ("w0", "b0", "g0", "be0", "m0", "v0", "wm", "bm", "gm", "bem",
           "mm", "vm", "wl", "bl", "gl", "bel", "ml", "vl", "bases")]
    )
    in_maps = []
    for n in range(8):
        xd, xrep = _prep_x(x[n])
        in_maps.append({"xd": xd, "xr": xrep, "wc": wc, "wg": wgm,
                        "bias": bias})
    res = run_bass_kernel_spmd(nc, in_maps, core_ids=list(range(8)))
    out = np.stack([r["out"] for r in res.results])
    return out.reshape(8, C * NB, H, W).astype(np.float32)
